# revision 29
# baseline (speedup 1.0000x reference)
"""GATv2 message-passing kernel v2 for 8 Trainium2 NeuronCores (Bass/Tile).

Strategy: edges sharded by RECEIVER ownership. Core c owns node blocks
[c*BPC, (c+1)*BPC), each block = 128 consecutive nodes. Host buckets every
edge to (block, tile-slot) with a uniform quota of T tiles per block
(static SPMD schedule). Per core:
  1. project all 50176 nodes -> DRAM table (for send-side gathers), and
     project the core's own 6272 nodes -> resident SBUF `blkres`
     (recv side, partitions = node-local index).
  2. stream edge tiles (128 edges each, batched C=4 per DVE/ACT op):
     - batched indirect gather of send projections table[s] -> gs
     - PE: transpose(rloc) -> rT, edge-feature projection (65-row lhsT
       folds We_bias), recv "gather" = onehotT @ blkres, and the
       segment-sum scatter = onehot @ payload accumulated in PSUM across
       the block's 26 tiles (start/stop flags)
     - DVE/ACT: onehot masks via is_equal, x = gs+recv+eproj,
       mish via Exp/Ln(1+u)/Tanh, logits, w=exp(logit), payload
     Pad slots have rloc=-1 -> all-zero onehot rows/cols -> contribute
     nothing anywhere.
  3. per block: divide numerator by denominator from the same PSUM acc,
     stage and write the core's output shard densely. No collectives.
Host assembles the 8 shards.
"""
import sys
import os

sys.path.insert(0, "/opt/trn_rl_repo")
import numpy as np
import concourse.bass as bass
import concourse.mybir as mybir
import concourse.tile as tile
import concourse.tile as tile_mod
from concourse.masks import make_identity
from concourse.vector_clock import ScopedClock

# ---------------------------------------------------------------------------
# Environment workarounds (same as baseline kernel.py):
# ---------------------------------------------------------------------------
try:
    from jax.interpreters import mlir as _mlir
    from concourse.bass2jax import (
        _bass_exec_p as _bep,
        _bass_exec_neuron_lowering as _benl,
        _partition_id_p as _pip,
        _partition_id_lowering as _pil,
    )

    _mlir.register_lowering(_bep, _benl, platform="axon")
    _mlir.register_lowering(_pip, _pil, platform="axon")
except Exception:  # pragma: no cover
    pass

_N_CARRIERS = 24


def _patched_drain_and_barrier(self, tick_clock, wait_clock):
    nc = self.nc
    nops = [nc.sync.nop(nofuse=True) for _ in range(_N_CARRIERS)]
    drain_inst = nc.sync.drain()
    wait_clock.add_sem_waits(
        drain_inst.ins, ScopedClock({None: tick_clock.global_clock}))
    waits = list(drain_inst.ins.sync_info.on_wait or [])
    if len(waits) > 1:
        assert len(waits) - 1 <= _N_CARRIERS
        drain_inst.ins.sync_info.on_wait = waits[:1]
        for nop, w in zip(nops, waits[1:]):
            si = nop.ins.sync_info
            if si is None:
                nop.ins.sync_info = mybir.SyncInfo(on_wait=[w], on_update=[])
            else:
                si.on_wait = [w]
    nc.all_engine_barrier()
    assert self.sems is not None
    popped = nc._tile_sem_poison_stack.pop()
    assert popped is self._sem_poison
    nc.clear_and_free_semaphores(list(self.sems.allocated().values()))
    nc.all_engine_barrier()


tile_mod.TileContext._drain_and_barrier = _patched_drain_and_barrier


def _split_excess_waits(nc, max_waits=1):
    for bbname, body in nc.bb_map.items():
        bb = body.bb
        insts = list(bb.instructions)
        out = []
        changed = False
        for ins in insts:
            si = ins.sync_info
            waits = list(si.on_wait) if si and si.on_wait else []
            if len(waits) > max_waits:
                keep = waits[:max_waits - 1] + [waits[-1]]
                extra = waits[max_waits - 1:-1]
                for w in extra:
                    nop = mybir.InstNoOp(
                        name=nc.get_next_instruction_name(), ins=[], outs=[])
                    nop.engine = ins.engine
                    nop.sync_info = mybir.SyncInfo(on_wait=[w], on_update=[])
                    nc.register_instruction(nop, overwrite=True)
                    out.append(nop)
                ins.sync_info.on_wait = keep
                changed = True
            out.append(ins)
        if changed:
            bb.instructions = out


F32 = mybir.dt.float32
BF16 = mybir.dt.bfloat16
I32 = mybir.dt.int32

N_NODES = 50000
N_EDGES = 1200000
IN_DIM = 128
EDGE_DIM = 64
EMBED = 64
HEADS = 8
PAY = EMBED + HEADS  # 72

MISH_MODE = os.environ.get("V2_MISH", "exp")      # exp | lntanh | native
GATHER_MODE = os.environ.get("V2_GATHER", "single")  # batch | single
NQ = int(os.environ.get("V2_NQ", "1"))            # SWDGE queues (1..4)
LG_BF16 = os.environ.get("V2_LGBF", "0") == "1"   # bf16 logits path


def _q(bi, q):
    if q:
        bi.ins.queue = f"qPoolDynamic{q}"
    return bi

N_CORES = 8
NPB = 128            # nodes per block (= onehot window width)
NPAD = 50176         # 392 * 128
NBLK = NPAD // NPB   # 392
BPC_FULL = NBLK // N_CORES  # 49 blocks per core
T_Q = 26             # quota: tiles per block (verified vs data in host_prep)
CBATCH = int(os.environ.get("V2_CBATCH", "4"))  # tiles per DVE/ACT op group
OB = 7               # blocks per output staging write


def _ap3(ap, mid_n):
    """[128, D] AP -> [128, mid_n(step0), D] broadcast view."""
    return bass.AP(ap.tensor, ap.offset, [ap.ap[0], [0, mid_n]] + list(ap.ap[1:]))


def _inner_b(ap, n):
    """Append a step-0 innermost free dim of size n (broadcast view)."""
    return bass.AP(ap.tensor, ap.offset, list(ap.ap) + [[0, n]])


def build_nc(bpc=BPC_FULL):
    tpc = bpc * T_Q            # tiles per core
    slots = tpc * 128
    nbat = (tpc + CBATCH - 1) // CBATCH
    ob = OB if bpc % OB == 0 else 1

    nc = bass.Bass(num_swdge_queues=NQ)

    nfT = nc.declare_dram_parameter("nfT", [IN_DIM, NPAD], F32, isOutput=False)
    nfTo = nc.declare_dram_parameter("nfTo", [IN_DIM, bpc * 128], F32, isOutput=False)
    eftT = nc.declare_dram_parameter("eftT", [EDGE_DIM + 1, slots], F32, isOutput=False)
    s_e = nc.declare_dram_parameter("s_idx", [128, tpc], I32, isOutput=False)
    r_e = nc.declare_dram_parameter("rloc", [128, tpc], F32, isOutput=False)
    W_e = nc.declare_dram_parameter("W", [IN_DIM, EMBED], F32, isOutput=False)
    Wb_e = nc.declare_dram_parameter("Wb", [128, EMBED], F32, isOutput=False)
    WeX_e = nc.declare_dram_parameter("WeX", [EDGE_DIM + 1, EMBED], F32, isOutput=False)
    aC_e = nc.declare_dram_parameter("aC", [128, CBATCH * EMBED],
                                     BF16 if LG_BF16 else F32, isOutput=False)
    iorow_e = nc.declare_dram_parameter("iorow", [128, 128], F32, isOutput=False)
    iocol_e = nc.declare_dram_parameter("iocol", [128, 128], F32, isOutput=False)
    out_e = nc.declare_dram_parameter("out_shard", [128, bpc, EMBED], F32, isOutput=True)

    table = nc.dram_tensor("ntable", [NPAD, EMBED], F32)

    with tile.TileContext(nc) as tc:
        with (
            tc.tile_pool(name="const", bufs=1) as cpool,
            tc.tile_pool(name="nproj", bufs=3) as npool,
            tc.tile_pool(name="edgea", bufs=4) as apool,
            tc.tile_pool(name="edgeb", bufs=3) as bpool,
            tc.tile_pool(name="fin", bufs=2) as fpool,
            tc.tile_pool(name="ps_n", bufs=1, space="PSUM") as ps_n,
            tc.tile_pool(name="ps_rt", bufs=2, space="PSUM") as ps_rt,
            tc.tile_pool(name="ps_gr", bufs=2, space="PSUM") as ps_gr,
            tc.tile_pool(name="ps_acc", bufs=2, space="PSUM") as ps_acc,
        ):
            # ---- constants
            W_t = cpool.tile([IN_DIM, EMBED], F32)
            nc.sync.dma_start(out=W_t[:], in_=W_e[:])
            Wb_t = cpool.tile([128, EMBED], F32)
            nc.sync.dma_start(out=Wb_t[:], in_=Wb_e[:])
            WeX_t = cpool.tile([EDGE_DIM + 1, EMBED], F32)
            nc.sync.dma_start(out=WeX_t[:], in_=WeX_e[:])
            aC_t = cpool.tile([128, CBATCH * EMBED], BF16 if LG_BF16 else F32)
            nc.sync.dma_start(out=aC_t[:], in_=aC_e[:])
            ior_t = cpool.tile([128, 128], F32)
            nc.sync.dma_start(out=ior_t[:], in_=iorow_e[:])
            ioc_t = cpool.tile([128, 128], F32)
            nc.sync.dma_start(out=ioc_t[:], in_=iocol_e[:])
            idt = cpool.tile([128, 128], F32)
            make_identity(nc, idt[:])
            # resident per-core recv projections and edge indices
            blkres = cpool.tile([128, bpc * EMBED], F32)
            s_t = cpool.tile([128, tpc], I32)
            nc.sync.dma_start(out=s_t[:], in_=s_e[:])
            rl_t = cpool.tile([128, tpc], F32)
            nc.sync.dma_start(out=rl_t[:], in_=r_e[:])

            # ---- phase 1a: project ALL nodes -> DRAM table (send side)
            tv = table[:].rearrange("(c p) d -> p c d", p=128)
            for g in range(NPAD // 1024):
                nf_t = npool.tile([IN_DIM, 1024], F32, tag="nf")
                nc.sync.dma_start(
                    out=nf_t[:], in_=nfT[:, g * 1024:(g + 1) * 1024])
                ps = ps_n.tile([128, 8, EMBED], F32, space="PSUM", tag="np")
                for j in range(8):
                    nc.tensor.matmul(
                        out=ps[:, j, :], lhsT=nf_t[:, j * 128:(j + 1) * 128],
                        rhs=W_t[:], start=True, stop=True)
                nb = npool.tile([128, 8, EMBED], F32, tag="nb")
                nc.vector.tensor_add(nb[:], ps[:], _ap3(Wb_t[:], 8))
                nc.sync.dma_start(out=tv[:, g * 8:(g + 1) * 8, :], in_=nb[:])

            # ---- phase 1b: project OWN nodes -> resident blkres (recv side)
            for g in range((bpc + 6) // 7):
                jw = min(7, bpc - g * 7)
                nf_t = npool.tile([IN_DIM, 896], F32, tag="nfo")
                nc.sync.dma_start(
                    out=nf_t[:, :jw * 128],
                    in_=nfTo[:, g * 896:g * 896 + jw * 128])
                ps = ps_n.tile([128, 8, EMBED], F32, space="PSUM", tag="np")
                for j in range(jw):
                    nc.tensor.matmul(
                        out=ps[:, j, :], lhsT=nf_t[:, j * 128:(j + 1) * 128],
                        rhs=W_t[:], start=True, stop=True)
                b0 = g * 7
                nc.vector.tensor_add(
                    blkres[:, b0 * EMBED:(b0 + jw) * EMBED].rearrange(
                        "p (c d) -> p c d", d=EMBED),
                    ps[:, :jw, :], _ap3(Wb_t[:], jw))

            # ---- phase 2: edge tiles, batched CBATCH at a time
            LOOKAHEAD = 3
            LGDT = BF16 if LG_BF16 else F32
            acc_tiles = {}
            ostage = {}

            def stage_a(bt):
                t0 = bt * CBATCH
                w = min(CBATCH, tpc - t0)
                ef_t = apool.tile([EDGE_DIM + 1, CBATCH * 128], F32, tag="eft")
                nc.sync.dma_start(
                    out=ef_t[:, :w * 128],
                    in_=eftT[:, t0 * 128:(t0 + w) * 128])
                gs = apool.tile([128, CBATCH, EMBED], F32, tag="gs")
                if GATHER_MODE == "batch":
                    nc.gpsimd.indirect_dma_start(
                        out=gs[:, :w, :], out_offset=None, in_=table[:],
                        in_offset=bass.IndirectOffsetOnAxis(
                            ap=s_t[:, t0:t0 + w], axis=0))
                else:
                    for j in range(w):
                        _q(nc.gpsimd.indirect_dma_start(
                            out=gs[:, j, :], out_offset=None, in_=table[:],
                            in_offset=bass.IndirectOffsetOnAxis(
                                ap=s_t[:, t0 + j:t0 + j + 1], axis=0)),
                           (t0 + j) % NQ)
                return ef_t, gs

            def stage_b(bt, ctx):
                ef_t, gs = ctx
                t0 = bt * CBATCH
                w = min(CBATCH, tpc - t0)
                psg = ps_gr.tile([128, CBATCH * EMBED], F32, space="PSUM", tag="gr")
                # onehot masks
                oh = bpool.tile([128, CBATCH, 128], F32, tag="oh")
                nc.vector.tensor_tensor(
                    out=oh[:, :w, :],
                    in0=_inner_b(rl_t[:, t0:t0 + w], 128),
                    in1=_ap3(ior_t[:], w),
                    op=mybir.AluOpType.is_equal)
                # rT transposes + transposed masks, in half-groups of 4
                ohT = bpool.tile([128, CBATCH, 128], F32, tag="ohT")
                for h0 in range(0, w, 4):
                    hw_ = min(4, w - h0)
                    psr = ps_rt.tile([128, 512], F32, space="PSUM", tag="rt")
                    for j in range(hw_):
                        nc.tensor.transpose(
                            out=psr[:, j * 128:(j + 1) * 128],
                            in_=rl_t[:, t0 + h0 + j:t0 + h0 + j + 1
                                     ].to_broadcast([128, 128]),
                            identity=idt[:])
                    nc.vector.tensor_tensor(
                        out=ohT[:, h0:h0 + hw_, :],
                        in0=_ap3(ioc_t[:], hw_),
                        in1=psr[:, :hw_ * 128].rearrange(
                            "p (c e) -> p c e", e=128),
                        op=mybir.AluOpType.is_equal)
                for j in range(w):
                    blk = (t0 + j) // T_Q
                    nc.tensor.matmul(
                        out=psg[:, j * EMBED:(j + 1) * EMBED],
                        lhsT=ef_t[:, j * 128:(j + 1) * 128],
                        rhs=WeX_t[:], start=True, stop=False)
                    nc.tensor.matmul(
                        out=psg[:, j * EMBED:(j + 1) * EMBED],
                        lhsT=ohT[:, j, :],
                        rhs=blkres[:, blk * EMBED:(blk + 1) * EMBED],
                        start=False, stop=True)
                # x = gs + (Grecv + eproj)
                x_t = bpool.tile([128, CBATCH * EMBED], F32, tag="x")
                nc.vector.tensor_add(
                    x_t[:, :w * EMBED],
                    gs[:, :w, :].rearrange("p c d -> p (c d)"),
                    psg[:, :w * EMBED])
                xm_t = bpool.tile([128, CBATCH * EMBED], LGDT, tag="xm")
                if MISH_MODE == "native":
                    nc.scalar.activation(xm_t[:, :w * EMBED], x_t[:, :w * EMBED],
                                         mybir.ActivationFunctionType.Mish)
                elif MISH_MODE == "lntanh":
                    # mish(x) = x * tanh(ln(1 + e^x))
                    u_t = bpool.tile([128, CBATCH * EMBED], F32, tag="mu")
                    nc.scalar.activation(u_t[:, :w * EMBED], x_t[:, :w * EMBED],
                                         mybir.ActivationFunctionType.Exp)
                    nc.scalar.activation(u_t[:, :w * EMBED], u_t[:, :w * EMBED],
                                         mybir.ActivationFunctionType.Ln, bias=1.0)
                    nc.scalar.activation(u_t[:, :w * EMBED], u_t[:, :w * EMBED],
                                         mybir.ActivationFunctionType.Tanh)
                    nc.vector.tensor_mul(
                        xm_t[:, :w * EMBED], x_t[:, :w * EMBED],
                        u_t[:, :w * EMBED])
                else:
                    # mish(x) = x * (u^2+2u)/(u^2+2u+2), u=e^x (exp-only)
                    u_t = bpool.tile([128, CBATCH * EMBED], F32, tag="mu")
                    tb_t = bpool.tile([128, CBATCH * EMBED], F32, tag="mtb")
                    nw = w * EMBED
                    nc.scalar.activation(u_t[:, :nw], x_t[:, :nw],
                                         mybir.ActivationFunctionType.Exp)
                    nc.vector.tensor_scalar_add(tb_t[:, :nw], u_t[:, :nw], 2.0)
                    nc.vector.tensor_mul(u_t[:, :nw], u_t[:, :nw], tb_t[:, :nw])
                    nc.vector.tensor_scalar_add(tb_t[:, :nw], u_t[:, :nw], 2.0)
                    nc.vector.reciprocal(tb_t[:, :nw], tb_t[:, :nw])
                    nc.vector.tensor_mul(u_t[:, :nw], u_t[:, :nw], tb_t[:, :nw])
                    nc.vector.tensor_mul(
                        xm_t[:, :nw], x_t[:, :nw], u_t[:, :nw])
                # logits and attention weights
                lg_t = bpool.tile([128, CBATCH * EMBED], LGDT, tag="lg")
                nc.vector.tensor_mul(
                    lg_t[:, :w * EMBED], xm_t[:, :w * EMBED],
                    aC_t[:, :w * EMBED])
                l_t = bpool.tile([128, CBATCH * HEADS], F32, tag="l")
                nc.vector.tensor_reduce(
                    l_t[:, :w * HEADS].rearrange("p (g o) -> p g o", o=1),
                    lg_t[:, :w * EMBED].rearrange("p (g i) -> p g i", i=8),
                    axis=mybir.AxisListType.X, op=mybir.AluOpType.add)
                w_t = bpool.tile([128, CBATCH * HEADS], F32, tag="w")
                nc.scalar.activation(w_t[:, :w * HEADS], l_t[:, :w * HEADS],
                                     mybir.ActivationFunctionType.Exp)
                # payload [w*gs ; w]
                pay_t = bpool.tile([128, CBATCH, PAY], F32, tag="pay")
                nc.vector.tensor_mul(
                    pay_t[:, :w, :EMBED].rearrange("p c (h o) -> p c h o", o=8),
                    gs[:, :w, :].rearrange("p c (h o) -> p c h o", o=8),
                    _inner_b(w_t[:, :w * HEADS].rearrange(
                        "p (c h) -> p c h", h=8), 8))
                nc.vector.tensor_copy(
                    pay_t[:, :w, EMBED:],
                    w_t[:, :w * HEADS].rearrange("p (c h) -> p c h", h=8))
                # segment-sum into per-block PSUM accumulator
                for j in range(w):
                    t = t0 + j
                    blk, tib = divmod(t, T_Q)
                    if tib == 0:
                        acc_tiles[blk] = ps_acc.tile(
                            [128, PAY], F32, space="PSUM", tag="acc",
                            name=f"acc{blk}")
                    nc.tensor.matmul(
                        out=acc_tiles[blk][:], lhsT=oh[:, j, :],
                        rhs=pay_t[:, j, :],
                        start=(tib == 0), stop=(tib == T_Q - 1))
                    if tib == T_Q - 1:
                        finalize(blk)

            def finalize(blk):
                acc = acc_tiles.pop(blk)
                og = blk // ob
                if og not in ostage:
                    ostage[og] = fpool.tile([128, ob, EMBED], F32, tag="ost",
                                            name=f"ost{og}")
                den = fpool.tile([128, HEADS], F32, tag="den")
                nc.vector.tensor_scalar_add(den[:], acc[:, EMBED:], 1e-30)
                rec = fpool.tile([128, HEADS], F32, tag="rec")
                nc.vector.reciprocal(rec[:], den[:])
                nc.vector.tensor_mul(
                    ostage[og][:, blk % ob, :].rearrange(
                        "p (h o) -> p h o", o=8),
                    acc[:, :EMBED].rearrange("p (h o) -> p h o", o=8),
                    _inner_b(rec[:], 8))
                if blk % ob == ob - 1:
                    st = ostage.pop(og)
                    nc.sync.dma_start(
                        out=out_e[:, og * ob:(og + 1) * ob, :], in_=st[:])

            pend = {}
            for bt in range(nbat):
                pend[bt] = stage_a(bt)
                if bt - LOOKAHEAD >= 0:
                    stage_b(bt - LOOKAHEAD, pend.pop(bt - LOOKAHEAD))
            for bt in sorted(pend):
                stage_b(bt, pend.pop(bt))

    _split_excess_waits(nc)
    return nc


def host_prep(node_features, edge_features, senders, receivers,
              W_kernel, W_bias, We_kernel, We_bias, a,
              n_cores=N_CORES, bpc=BPC_FULL):
    """Pure layout transforms -> per-core input maps."""
    tpc = bpc * T_Q
    slots = tpc * 128
    nodes_pc = bpc * 128

    senders = np.asarray(senders, np.int32)
    receivers = np.asarray(receivers, np.int32)
    edge_features = np.asarray(edge_features, np.float32)

    nf_pad = np.zeros((NPAD, IN_DIM), np.float32)
    nf_pad[:N_NODES] = np.asarray(node_features, np.float32)
    nfT = np.ascontiguousarray(nf_pad.T)

    # bucket edges by receiver block; quota T_Q tiles per block
    blk = receivers // NPB
    order = np.argsort(blk, kind="stable")
    blk_sorted = blk[order]
    counts = np.bincount(blk_sorted, minlength=NBLK)
    assert ((counts + 127) // 128).max() <= T_Q, "quota exceeded"
    starts = np.zeros(NBLK + 1, np.int64)
    np.cumsum(counts, out=starts[1:])
    rank = np.arange(len(order)) - starts[blk_sorted]  # rank within block

    core_of = blk_sorted // bpc          # owning core per sorted edge
    col_in_core = (blk_sorted % bpc) * T_Q + rank // 128
    row = rank % 128
    flat = col_in_core * 128 + row       # slot within core buffers

    keep = core_of < n_cores             # (all true in full mode)
    e_ids = order[keep]
    core_of, flat = core_of[keep], flat[keep]

    a_flat = np.asarray(a, np.float32).reshape(-1)
    aC = np.tile(a_flat[None, :], (128, CBATCH))
    if LG_BF16:
        aC = aC.astype(mybir.dt.np(BF16))
    iorow = np.tile(np.arange(128, dtype=np.float32)[None, :], (128, 1))
    iocol = np.tile(np.arange(128, dtype=np.float32)[:, None], (1, 128))
    WeX = np.concatenate([np.asarray(We_kernel, np.float32),
                          np.asarray(We_bias, np.float32)[None, :]], 0)
    Wb_rep = np.tile(np.asarray(W_bias, np.float32)[None, :], (128, 1))

    in_maps = []
    for c in range(n_cores):
        m = core_of == c
        fl = flat[m]
        eid = e_ids[m]
        s_buf = np.zeros(slots, np.int32)
        r_buf = np.full(slots, -1.0, np.float32)
        ef_buf = np.zeros((slots, EDGE_DIM + 1), np.float32)
        s_buf[fl] = senders[eid]
        r_buf[fl] = (receivers[eid] % NPB).astype(np.float32)
        ef_buf[fl, :EDGE_DIM] = edge_features[eid]
        ef_buf[fl, EDGE_DIM] = 1.0
        n0 = c * nodes_pc
        in_maps.append({
            "nfT": nfT,
            "nfTo": np.ascontiguousarray(nfT[:, n0:n0 + nodes_pc]),
            "eftT": np.ascontiguousarray(ef_buf.T),
            "s_idx": np.ascontiguousarray(
                s_buf.reshape(tpc, 128).T),
            "rloc": np.ascontiguousarray(
                r_buf.reshape(tpc, 128).T),
            "W": np.asarray(W_kernel, np.float32),
            "Wb": Wb_rep,
            "WeX": WeX,
            "aC": aC,
            "iorow": iorow,
            "iocol": iocol,
        })
    return in_maps


def _build_runner(nc, n_cores):
    """Jitted 8-core SPMD executor via the axon PJRT tunnel (shard_map)."""
    import time
    import jax
    from jax.sharding import Mesh, PartitionSpec
    from jax.experimental.shard_map import shard_map
    from concourse import bass2jax
    from concourse.bass2jax import _bass_exec_p, install_neuronx_cc_hook

    install_neuronx_cc_hook()
    partition_name = nc.partition_id_tensor.name if nc.partition_id_tensor else None
    in_names, out_names, out_avals, zero_outs = [], [], [], []
    for alloc in nc.m.functions[0].allocations:
        if not isinstance(alloc, mybir.MemoryLocationSet):
            continue
        name = alloc.memorylocations[0].name
        if alloc.kind == "ExternalInput":
            if name != partition_name:
                in_names.append(name)
        elif alloc.kind == "ExternalOutput":
            out_names.append(name)
            shape = tuple(alloc.tensor_shape)
            dtype = mybir.dt.np(alloc.dtype)
            out_avals.append(jax.core.ShapedArray(shape, dtype))
            zero_outs.append(np.zeros(shape, dtype))
    n_params = len(in_names)
    n_outs = len(out_avals)
    all_in_names = list(in_names) + list(out_names)
    if partition_name is not None:
        all_in_names.append(partition_name)

    def _body(*args):
        operands = list(args)
        if partition_name is not None:
            operands.append(bass2jax.partition_id_tensor())
        return tuple(_bass_exec_p.bind(
            *operands,
            out_avals=tuple(out_avals),
            in_names=tuple(all_in_names),
            out_names=tuple(out_names),
            lowering_input_output_aliases=(),
            sim_require_finite=True,
            sim_require_nnan=True,
            nc=nc,
        ))

    donate = tuple(range(n_params, n_params + n_outs))
    devices = jax.devices()[:n_cores]
    mesh = Mesh(np.asarray(devices), ("core",))
    in_specs = (PartitionSpec("core"),) * (n_params + n_outs)
    out_specs = (PartitionSpec("core"),) * len(out_names)
    jfn = jax.jit(
        shard_map(_body, mesh=mesh, in_specs=in_specs, out_specs=out_specs,
                  check_rep=False),
        donate_argnums=donate, keep_unused=True)

    def fn(in_maps):
        concat_in = [
            np.concatenate([np.asarray(in_maps[c][n]) for c in range(n_cores)], 0)
            for n in in_names
        ]
        concat_zeros = [np.zeros((n_cores * z.shape[0], *z.shape[1:]), z.dtype)
                        for z in zero_outs]
        t0 = time.perf_counter()
        out_arrs = jfn(*concat_in, *concat_zeros)
        out_arrs = [np.asarray(o) for o in out_arrs]
        dt = time.perf_counter() - t0
        return [
            {n: out_arrs[i].reshape(n_cores, *out_avals[i].shape)[c]
             for i, n in enumerate(out_names)}
            for c in range(n_cores)
        ], dt

    return fn


_CACHE = {}


def kernel(node_features, edge_features, global_features, senders, receivers,
           W_kernel, W_bias, We_kernel, We_bias, a):
    node_features = np.asarray(node_features, np.float32)
    edge_features = np.asarray(edge_features, np.float32)
    senders = np.asarray(senders, np.int32)
    receivers = np.asarray(receivers, np.int32)
    in_maps = host_prep(node_features, edge_features, senders, receivers,
                        W_kernel, W_bias, We_kernel, We_bias, a)
    if "fn" not in _CACHE:
        nc = build_nc()
        _CACHE["fn"] = _build_runner(nc, N_CORES)
    res, dt = _CACHE["fn"](in_maps)
    _CACHE["last_dt"] = dt
    # out_shard [128, bpc, 64]; global row = (c*bpc + b)*128 + p
    full = np.concatenate(
        [r["out_shard"].transpose(1, 0, 2).reshape(BPC_FULL * 128, EMBED)
         for r in res], axis=0)
    return full[:N_NODES].astype(np.float32)


# revision 36
# speedup vs baseline: 1.0003x; 1.0003x over previous
"""GATv2 message-passing kernel v2 for 8 Trainium2 NeuronCores (Bass/Tile).

Strategy: edges sharded by RECEIVER ownership. Core c owns node blocks
[c*BPC, (c+1)*BPC), each block = 128 consecutive nodes. Host buckets every
edge to (block, tile-slot) with a uniform quota of T tiles per block
(static SPMD schedule). Per core:
  1. project all 50176 nodes -> DRAM table (for send-side gathers), and
     project the core's own 6272 nodes -> resident SBUF `blkres`
     (recv side, partitions = node-local index).
  2. stream edge tiles (128 edges each, batched C=4 per DVE/ACT op):
     - batched indirect gather of send projections table[s] -> gs
     - PE: transpose(rloc) -> rT, edge-feature projection (65-row lhsT
       folds We_bias), recv "gather" = onehotT @ blkres, and the
       segment-sum scatter = onehot @ payload accumulated in PSUM across
       the block's 26 tiles (start/stop flags)
     - DVE/ACT: onehot masks via is_equal, x = gs+recv+eproj,
       mish via Exp/Ln(1+u)/Tanh, logits, w=exp(logit), payload
     Pad slots have rloc=-1 -> all-zero onehot rows/cols -> contribute
     nothing anywhere.
  3. per block: divide numerator by denominator from the same PSUM acc,
     stage and write the core's output shard densely. No collectives.
Host assembles the 8 shards.
"""
import sys
import os

sys.path.insert(0, "/opt/trn_rl_repo")
import numpy as np
import concourse.bass as bass
import concourse.mybir as mybir
import concourse.tile as tile
import concourse.tile as tile_mod
from concourse.masks import make_identity
from concourse.vector_clock import ScopedClock

# ---------------------------------------------------------------------------
# Environment workarounds (same as baseline kernel.py):
# ---------------------------------------------------------------------------
try:
    from jax.interpreters import mlir as _mlir
    from concourse.bass2jax import (
        _bass_exec_p as _bep,
        _bass_exec_neuron_lowering as _benl,
        _partition_id_p as _pip,
        _partition_id_lowering as _pil,
    )

    _mlir.register_lowering(_bep, _benl, platform="axon")
    _mlir.register_lowering(_pip, _pil, platform="axon")
except Exception:  # pragma: no cover
    pass

_N_CARRIERS = 24


def _patched_drain_and_barrier(self, tick_clock, wait_clock):
    nc = self.nc
    nops = [nc.sync.nop(nofuse=True) for _ in range(_N_CARRIERS)]
    drain_inst = nc.sync.drain()
    wait_clock.add_sem_waits(
        drain_inst.ins, ScopedClock({None: tick_clock.global_clock}))
    waits = list(drain_inst.ins.sync_info.on_wait or [])
    if len(waits) > 1:
        assert len(waits) - 1 <= _N_CARRIERS
        drain_inst.ins.sync_info.on_wait = waits[:1]
        for nop, w in zip(nops, waits[1:]):
            si = nop.ins.sync_info
            if si is None:
                nop.ins.sync_info = mybir.SyncInfo(on_wait=[w], on_update=[])
            else:
                si.on_wait = [w]
    nc.all_engine_barrier()
    assert self.sems is not None
    popped = nc._tile_sem_poison_stack.pop()
    assert popped is self._sem_poison
    nc.clear_and_free_semaphores(list(self.sems.allocated().values()))
    nc.all_engine_barrier()


tile_mod.TileContext._drain_and_barrier = _patched_drain_and_barrier


def _split_excess_waits(nc, max_waits=1):
    for bbname, body in nc.bb_map.items():
        bb = body.bb
        insts = list(bb.instructions)
        out = []
        changed = False
        for ins in insts:
            si = ins.sync_info
            waits = list(si.on_wait) if si and si.on_wait else []
            if len(waits) > max_waits:
                keep = waits[:max_waits - 1] + [waits[-1]]
                extra = waits[max_waits - 1:-1]
                for w in extra:
                    nop = mybir.InstNoOp(
                        name=nc.get_next_instruction_name(), ins=[], outs=[])
                    nop.engine = ins.engine
                    nop.sync_info = mybir.SyncInfo(on_wait=[w], on_update=[])
                    nc.register_instruction(nop, overwrite=True)
                    out.append(nop)
                ins.sync_info.on_wait = keep
                changed = True
            out.append(ins)
        if changed:
            bb.instructions = out


F32 = mybir.dt.float32
BF16 = mybir.dt.bfloat16
I32 = mybir.dt.int32

N_NODES = 50000
N_EDGES = 1200000
IN_DIM = 128
EDGE_DIM = 64
EMBED = 64
HEADS = 8
PAY = EMBED + HEADS  # 72

MISH_MODE = os.environ.get("V2_MISH", "exp")      # exp | lntanh | native
GATHER_MODE = os.environ.get("V2_GATHER", "single")  # batch | single
NQ = int(os.environ.get("V2_NQ", "1"))            # SWDGE queues (1..4)
LG_BF16 = os.environ.get("V2_LGBF", "0") == "1"   # bf16 logits path
MISH_BF16 = os.environ.get("V2_MISHBF", "0") == "1"  # bf16 mish chain
LOOKAHEAD = int(os.environ.get("V2_LOOKA", "3"))  # gather lookahead batches


def _q(bi, q):
    if q:
        bi.ins.queue = f"qPoolDynamic{q}"
    return bi

N_CORES = 8
NPB = 128            # nodes per block (= onehot window width)
NPAD = 50176         # 392 * 128
NBLK = NPAD // NPB   # 392
BPC_FULL = NBLK // N_CORES  # 49 blocks per core
T_Q = 26             # quota: tiles per block (verified vs data in host_prep)
CBATCH = int(os.environ.get("V2_CBATCH", "4"))  # tiles per DVE/ACT op group
OB = 7               # blocks per output staging write


def _ap3(ap, mid_n):
    """[128, D] AP -> [128, mid_n(step0), D] broadcast view."""
    return bass.AP(ap.tensor, ap.offset, [ap.ap[0], [0, mid_n]] + list(ap.ap[1:]))


def _inner_b(ap, n):
    """Append a step-0 innermost free dim of size n (broadcast view)."""
    return bass.AP(ap.tensor, ap.offset, list(ap.ap) + [[0, n]])


def build_nc(bpc=BPC_FULL):
    tpc = bpc * T_Q            # tiles per core
    slots = tpc * 128
    nbat = (tpc + CBATCH - 1) // CBATCH
    ob = OB if bpc % OB == 0 else 1

    nc = bass.Bass(num_swdge_queues=NQ)

    nfT = nc.declare_dram_parameter("nfT", [IN_DIM, NPAD], F32, isOutput=False)
    nfTo = nc.declare_dram_parameter("nfTo", [IN_DIM, bpc * 128], F32, isOutput=False)
    eftT = nc.declare_dram_parameter("eftT", [EDGE_DIM + 1, slots], F32, isOutput=False)
    s_e = nc.declare_dram_parameter("s_idx", [128, tpc], I32, isOutput=False)
    r_e = nc.declare_dram_parameter("rloc", [128, tpc], F32, isOutput=False)
    W_e = nc.declare_dram_parameter("W", [IN_DIM, EMBED], F32, isOutput=False)
    Wb_e = nc.declare_dram_parameter("Wb", [128, EMBED], F32, isOutput=False)
    WeX_e = nc.declare_dram_parameter("WeX", [EDGE_DIM + 1, EMBED], F32, isOutput=False)
    aC_e = nc.declare_dram_parameter("aC", [128, CBATCH * EMBED],
                                     BF16 if LG_BF16 else F32, isOutput=False)
    iorow_e = nc.declare_dram_parameter("iorow", [128, 128], F32, isOutput=False)
    iocol_e = nc.declare_dram_parameter("iocol", [128, 128], F32, isOutput=False)
    out_e = nc.declare_dram_parameter("out_shard", [128, bpc, EMBED], F32, isOutput=True)

    table = nc.dram_tensor("ntable", [NPAD, EMBED], F32)

    with tile.TileContext(nc) as tc:
        with (
            tc.tile_pool(name="const", bufs=1) as cpool,
            tc.tile_pool(name="nproj", bufs=3) as npool,
            tc.tile_pool(name="edgea", bufs=LOOKAHEAD + 1) as apool,
            tc.tile_pool(name="edgeb", bufs=3) as bpool,
            tc.tile_pool(name="fin", bufs=2) as fpool,
            tc.tile_pool(name="ps_n", bufs=1, space="PSUM") as ps_n,
            tc.tile_pool(name="ps_rt", bufs=2, space="PSUM") as ps_rt,
            tc.tile_pool(name="ps_gr", bufs=2, space="PSUM") as ps_gr,
            tc.tile_pool(name="ps_acc", bufs=2, space="PSUM") as ps_acc,
        ):
            # ---- constants
            W_t = cpool.tile([IN_DIM, EMBED], F32)
            nc.sync.dma_start(out=W_t[:], in_=W_e[:])
            Wb_t = cpool.tile([128, EMBED], F32)
            nc.sync.dma_start(out=Wb_t[:], in_=Wb_e[:])
            WeX_t = cpool.tile([EDGE_DIM + 1, EMBED], F32)
            nc.sync.dma_start(out=WeX_t[:], in_=WeX_e[:])
            aC_t = cpool.tile([128, CBATCH * EMBED], BF16 if LG_BF16 else F32)
            nc.sync.dma_start(out=aC_t[:], in_=aC_e[:])
            ior_t = cpool.tile([128, 128], F32)
            nc.sync.dma_start(out=ior_t[:], in_=iorow_e[:])
            ioc_t = cpool.tile([128, 128], F32)
            nc.sync.dma_start(out=ioc_t[:], in_=iocol_e[:])
            idt = cpool.tile([128, 128], F32)
            make_identity(nc, idt[:])
            # resident per-core recv projections and edge indices
            blkres = cpool.tile([128, bpc * EMBED], F32)
            s_t = cpool.tile([128, tpc], I32)
            nc.sync.dma_start(out=s_t[:], in_=s_e[:])
            rl_t = cpool.tile([128, tpc], F32)
            nc.sync.dma_start(out=rl_t[:], in_=r_e[:])

            # ---- phase 1a: project ALL nodes -> DRAM table (send side)
            tv = table[:].rearrange("(c p) d -> p c d", p=128)
            for g in range(NPAD // 1024):
                nf_t = npool.tile([IN_DIM, 1024], F32, tag="nf")
                nc.sync.dma_start(
                    out=nf_t[:], in_=nfT[:, g * 1024:(g + 1) * 1024])
                ps = ps_n.tile([128, 8, EMBED], F32, space="PSUM", tag="np")
                for j in range(8):
                    nc.tensor.matmul(
                        out=ps[:, j, :], lhsT=nf_t[:, j * 128:(j + 1) * 128],
                        rhs=W_t[:], start=True, stop=True)
                nb = npool.tile([128, 8, EMBED], F32, tag="nb")
                nc.vector.tensor_add(nb[:], ps[:], _ap3(Wb_t[:], 8))
                nc.sync.dma_start(out=tv[:, g * 8:(g + 1) * 8, :], in_=nb[:])

            # ---- phase 1b: project OWN nodes -> resident blkres (recv side)
            for g in range((bpc + 6) // 7):
                jw = min(7, bpc - g * 7)
                nf_t = npool.tile([IN_DIM, 896], F32, tag="nfo")
                nc.sync.dma_start(
                    out=nf_t[:, :jw * 128],
                    in_=nfTo[:, g * 896:g * 896 + jw * 128])
                ps = ps_n.tile([128, 8, EMBED], F32, space="PSUM", tag="np")
                for j in range(jw):
                    nc.tensor.matmul(
                        out=ps[:, j, :], lhsT=nf_t[:, j * 128:(j + 1) * 128],
                        rhs=W_t[:], start=True, stop=True)
                b0 = g * 7
                nc.vector.tensor_add(
                    blkres[:, b0 * EMBED:(b0 + jw) * EMBED].rearrange(
                        "p (c d) -> p c d", d=EMBED),
                    ps[:, :jw, :], _ap3(Wb_t[:], jw))

            # ---- phase 2: edge tiles, batched CBATCH at a time
            LGDT = BF16 if LG_BF16 else F32
            MDT = BF16 if MISH_BF16 else F32
            acc_tiles = {}
            ostage = {}

            def stage_a(bt):
                t0 = bt * CBATCH
                w = min(CBATCH, tpc - t0)
                ef_t = apool.tile([EDGE_DIM + 1, CBATCH * 128], F32, tag="eft")
                nc.sync.dma_start(
                    out=ef_t[:, :w * 128],
                    in_=eftT[:, t0 * 128:(t0 + w) * 128])
                gs = apool.tile([128, CBATCH, EMBED], F32, tag="gs")
                if GATHER_MODE == "batch":
                    nc.gpsimd.indirect_dma_start(
                        out=gs[:, :w, :], out_offset=None, in_=table[:],
                        in_offset=bass.IndirectOffsetOnAxis(
                            ap=s_t[:, t0:t0 + w], axis=0))
                else:
                    for j in range(w):
                        _q(nc.gpsimd.indirect_dma_start(
                            out=gs[:, j, :], out_offset=None, in_=table[:],
                            in_offset=bass.IndirectOffsetOnAxis(
                                ap=s_t[:, t0 + j:t0 + j + 1], axis=0)),
                           (t0 + j) % NQ)
                return ef_t, gs

            def stage_b(bt, ctx):
                ef_t, gs = ctx
                t0 = bt * CBATCH
                w = min(CBATCH, tpc - t0)
                psg = ps_gr.tile([128, CBATCH * EMBED], F32, space="PSUM", tag="gr")
                # onehot masks
                oh = bpool.tile([128, CBATCH, 128], F32, tag="oh")
                nc.vector.tensor_tensor(
                    out=oh[:, :w, :],
                    in0=_inner_b(rl_t[:, t0:t0 + w], 128),
                    in1=_ap3(ior_t[:], w),
                    op=mybir.AluOpType.is_equal)
                # rT transposes + transposed masks, in half-groups of 4
                ohT = bpool.tile([128, CBATCH, 128], F32, tag="ohT")
                for h0 in range(0, w, 4):
                    hw_ = min(4, w - h0)
                    psr = ps_rt.tile([128, 512], F32, space="PSUM", tag="rt")
                    for j in range(hw_):
                        nc.tensor.transpose(
                            out=psr[:, j * 128:(j + 1) * 128],
                            in_=rl_t[:, t0 + h0 + j:t0 + h0 + j + 1
                                     ].to_broadcast([128, 128]),
                            identity=idt[:])
                    nc.vector.tensor_tensor(
                        out=ohT[:, h0:h0 + hw_, :],
                        in0=_ap3(ioc_t[:], hw_),
                        in1=psr[:, :hw_ * 128].rearrange(
                            "p (c e) -> p c e", e=128),
                        op=mybir.AluOpType.is_equal)
                for j in range(w):
                    blk = (t0 + j) // T_Q
                    nc.tensor.matmul(
                        out=psg[:, j * EMBED:(j + 1) * EMBED],
                        lhsT=ef_t[:, j * 128:(j + 1) * 128],
                        rhs=WeX_t[:], start=True, stop=False)
                    nc.tensor.matmul(
                        out=psg[:, j * EMBED:(j + 1) * EMBED],
                        lhsT=ohT[:, j, :],
                        rhs=blkres[:, blk * EMBED:(blk + 1) * EMBED],
                        start=False, stop=True)
                # x = gs + (Grecv + eproj)
                x_t = bpool.tile([128, CBATCH * EMBED], MDT, tag="x")
                nc.vector.tensor_add(
                    x_t[:, :w * EMBED],
                    gs[:, :w, :].rearrange("p c d -> p (c d)"),
                    psg[:, :w * EMBED])
                xm_t = bpool.tile([128, CBATCH * EMBED], LGDT, tag="xm")
                if MISH_MODE == "native":
                    nc.scalar.activation(xm_t[:, :w * EMBED], x_t[:, :w * EMBED],
                                         mybir.ActivationFunctionType.Mish)
                elif MISH_MODE == "lntanh":
                    # mish(x) = x * tanh(ln(1 + e^x))
                    u_t = bpool.tile([128, CBATCH * EMBED], F32, tag="mu")
                    nc.scalar.activation(u_t[:, :w * EMBED], x_t[:, :w * EMBED],
                                         mybir.ActivationFunctionType.Exp)
                    nc.scalar.activation(u_t[:, :w * EMBED], u_t[:, :w * EMBED],
                                         mybir.ActivationFunctionType.Ln, bias=1.0)
                    nc.scalar.activation(u_t[:, :w * EMBED], u_t[:, :w * EMBED],
                                         mybir.ActivationFunctionType.Tanh)
                    nc.vector.tensor_mul(
                        xm_t[:, :w * EMBED], x_t[:, :w * EMBED],
                        u_t[:, :w * EMBED])
                else:
                    # mish(x) = x * (u^2+2u)/(u^2+2u+2), u=e^x (exp-only)
                    u_t = bpool.tile([128, CBATCH * EMBED], MDT, tag="mu")
                    tb_t = bpool.tile([128, CBATCH * EMBED], MDT, tag="mtb")
                    nw = w * EMBED
                    nc.scalar.activation(u_t[:, :nw], x_t[:, :nw],
                                         mybir.ActivationFunctionType.Exp)
                    nc.vector.tensor_scalar_add(tb_t[:, :nw], u_t[:, :nw], 2.0)
                    nc.vector.tensor_mul(u_t[:, :nw], u_t[:, :nw], tb_t[:, :nw])
                    nc.vector.tensor_scalar_add(tb_t[:, :nw], u_t[:, :nw], 2.0)
                    with nc.allow_low_precision(reason="mish frac recip"):
                        nc.vector.reciprocal(tb_t[:, :nw], tb_t[:, :nw])
                    nc.vector.tensor_mul(u_t[:, :nw], u_t[:, :nw], tb_t[:, :nw])
                    nc.vector.tensor_mul(
                        xm_t[:, :nw], x_t[:, :nw], u_t[:, :nw])
                # logits and attention weights
                lg_t = bpool.tile([128, CBATCH * EMBED], LGDT, tag="lg")
                nc.vector.tensor_mul(
                    lg_t[:, :w * EMBED], xm_t[:, :w * EMBED],
                    aC_t[:, :w * EMBED])
                l_t = bpool.tile([128, CBATCH * HEADS], F32, tag="l")
                with nc.allow_low_precision(reason="8-wide logit reduce"):
                    nc.vector.tensor_reduce(
                        l_t[:, :w * HEADS].rearrange("p (g o) -> p g o", o=1),
                        lg_t[:, :w * EMBED].rearrange("p (g i) -> p g i", i=8),
                        axis=mybir.AxisListType.X, op=mybir.AluOpType.add)
                w_t = bpool.tile([128, CBATCH * HEADS], F32, tag="w")
                nc.scalar.activation(w_t[:, :w * HEADS], l_t[:, :w * HEADS],
                                     mybir.ActivationFunctionType.Exp)
                # payload [w*gs ; w]
                pay_t = bpool.tile([128, CBATCH, PAY], F32, tag="pay")
                nc.vector.tensor_mul(
                    pay_t[:, :w, :EMBED].rearrange("p c (h o) -> p c h o", o=8),
                    gs[:, :w, :].rearrange("p c (h o) -> p c h o", o=8),
                    _inner_b(w_t[:, :w * HEADS].rearrange(
                        "p (c h) -> p c h", h=8), 8))
                nc.vector.tensor_copy(
                    pay_t[:, :w, EMBED:],
                    w_t[:, :w * HEADS].rearrange("p (c h) -> p c h", h=8))
                # segment-sum into per-block PSUM accumulator
                for j in range(w):
                    t = t0 + j
                    blk, tib = divmod(t, T_Q)
                    if tib == 0:
                        acc_tiles[blk] = ps_acc.tile(
                            [128, PAY], F32, space="PSUM", tag="acc",
                            name=f"acc{blk}")
                    nc.tensor.matmul(
                        out=acc_tiles[blk][:], lhsT=oh[:, j, :],
                        rhs=pay_t[:, j, :],
                        start=(tib == 0), stop=(tib == T_Q - 1))
                    if tib == T_Q - 1:
                        finalize(blk)

            def finalize(blk):
                acc = acc_tiles.pop(blk)
                og = blk // ob
                if og not in ostage:
                    ostage[og] = fpool.tile([128, ob, EMBED], F32, tag="ost",
                                            name=f"ost{og}")
                den = fpool.tile([128, HEADS], F32, tag="den")
                nc.vector.tensor_scalar_add(den[:], acc[:, EMBED:], 1e-30)
                rec = fpool.tile([128, HEADS], F32, tag="rec")
                nc.vector.reciprocal(rec[:], den[:])
                nc.vector.tensor_mul(
                    ostage[og][:, blk % ob, :].rearrange(
                        "p (h o) -> p h o", o=8),
                    acc[:, :EMBED].rearrange("p (h o) -> p h o", o=8),
                    _inner_b(rec[:], 8))
                if blk % ob == ob - 1:
                    st = ostage.pop(og)
                    nc.sync.dma_start(
                        out=out_e[:, og * ob:(og + 1) * ob, :], in_=st[:])

            pend = {}
            for bt in range(nbat):
                pend[bt] = stage_a(bt)
                if bt - LOOKAHEAD >= 0:
                    stage_b(bt - LOOKAHEAD, pend.pop(bt - LOOKAHEAD))
            for bt in sorted(pend):
                stage_b(bt, pend.pop(bt))

    _split_excess_waits(nc)
    return nc


def host_prep(node_features, edge_features, senders, receivers,
              W_kernel, W_bias, We_kernel, We_bias, a,
              n_cores=N_CORES, bpc=BPC_FULL):
    """Pure layout transforms -> per-core input maps."""
    tpc = bpc * T_Q
    slots = tpc * 128
    nodes_pc = bpc * 128

    senders = np.asarray(senders, np.int32)
    receivers = np.asarray(receivers, np.int32)
    edge_features = np.asarray(edge_features, np.float32)

    nf_pad = np.zeros((NPAD, IN_DIM), np.float32)
    nf_pad[:N_NODES] = np.asarray(node_features, np.float32)
    nfT = np.ascontiguousarray(nf_pad.T)

    # bucket edges by receiver block; quota T_Q tiles per block
    blk = receivers // NPB
    order = np.argsort(blk, kind="stable")
    blk_sorted = blk[order]
    counts = np.bincount(blk_sorted, minlength=NBLK)
    assert ((counts + 127) // 128).max() <= T_Q, "quota exceeded"
    starts = np.zeros(NBLK + 1, np.int64)
    np.cumsum(counts, out=starts[1:])
    rank = np.arange(len(order)) - starts[blk_sorted]  # rank within block

    core_of = blk_sorted // bpc          # owning core per sorted edge
    col_in_core = (blk_sorted % bpc) * T_Q + rank // 128
    row = rank % 128
    flat = col_in_core * 128 + row       # slot within core buffers

    keep = core_of < n_cores             # (all true in full mode)
    e_ids = order[keep]
    core_of, flat = core_of[keep], flat[keep]

    a_flat = np.asarray(a, np.float32).reshape(-1)
    aC = np.tile(a_flat[None, :], (128, CBATCH))
    if LG_BF16:
        aC = aC.astype(mybir.dt.np(BF16))
    iorow = np.tile(np.arange(128, dtype=np.float32)[None, :], (128, 1))
    iocol = np.tile(np.arange(128, dtype=np.float32)[:, None], (1, 128))
    WeX = np.concatenate([np.asarray(We_kernel, np.float32),
                          np.asarray(We_bias, np.float32)[None, :]], 0)
    Wb_rep = np.tile(np.asarray(W_bias, np.float32)[None, :], (128, 1))

    in_maps = []
    for c in range(n_cores):
        m = core_of == c
        fl = flat[m]
        eid = e_ids[m]
        s_buf = np.zeros(slots, np.int32)
        r_buf = np.full(slots, -1.0, np.float32)
        ef_buf = np.zeros((slots, EDGE_DIM + 1), np.float32)
        s_buf[fl] = senders[eid]
        r_buf[fl] = (receivers[eid] % NPB).astype(np.float32)
        ef_buf[fl, :EDGE_DIM] = edge_features[eid]
        ef_buf[fl, EDGE_DIM] = 1.0
        n0 = c * nodes_pc
        in_maps.append({
            "nfT": nfT,
            "nfTo": np.ascontiguousarray(nfT[:, n0:n0 + nodes_pc]),
            "eftT": np.ascontiguousarray(ef_buf.T),
            "s_idx": np.ascontiguousarray(
                s_buf.reshape(tpc, 128).T),
            "rloc": np.ascontiguousarray(
                r_buf.reshape(tpc, 128).T),
            "W": np.asarray(W_kernel, np.float32),
            "Wb": Wb_rep,
            "WeX": WeX,
            "aC": aC,
            "iorow": iorow,
            "iocol": iocol,
        })
    return in_maps


def _build_runner(nc, n_cores):
    """Jitted 8-core SPMD executor via the axon PJRT tunnel (shard_map)."""
    import time
    import jax
    from jax.sharding import Mesh, PartitionSpec
    from jax.experimental.shard_map import shard_map
    from concourse import bass2jax
    from concourse.bass2jax import _bass_exec_p, install_neuronx_cc_hook

    install_neuronx_cc_hook()
    partition_name = nc.partition_id_tensor.name if nc.partition_id_tensor else None
    in_names, out_names, out_avals, zero_outs = [], [], [], []
    for alloc in nc.m.functions[0].allocations:
        if not isinstance(alloc, mybir.MemoryLocationSet):
            continue
        name = alloc.memorylocations[0].name
        if alloc.kind == "ExternalInput":
            if name != partition_name:
                in_names.append(name)
        elif alloc.kind == "ExternalOutput":
            out_names.append(name)
            shape = tuple(alloc.tensor_shape)
            dtype = mybir.dt.np(alloc.dtype)
            out_avals.append(jax.core.ShapedArray(shape, dtype))
            zero_outs.append(np.zeros(shape, dtype))
    n_params = len(in_names)
    n_outs = len(out_avals)
    all_in_names = list(in_names) + list(out_names)
    if partition_name is not None:
        all_in_names.append(partition_name)

    def _body(*args):
        operands = list(args)
        if partition_name is not None:
            operands.append(bass2jax.partition_id_tensor())
        return tuple(_bass_exec_p.bind(
            *operands,
            out_avals=tuple(out_avals),
            in_names=tuple(all_in_names),
            out_names=tuple(out_names),
            lowering_input_output_aliases=(),
            sim_require_finite=True,
            sim_require_nnan=True,
            nc=nc,
        ))

    donate = tuple(range(n_params, n_params + n_outs))
    devices = jax.devices()[:n_cores]
    mesh = Mesh(np.asarray(devices), ("core",))
    in_specs = (PartitionSpec("core"),) * (n_params + n_outs)
    out_specs = (PartitionSpec("core"),) * len(out_names)
    jfn = jax.jit(
        shard_map(_body, mesh=mesh, in_specs=in_specs, out_specs=out_specs,
                  check_rep=False),
        donate_argnums=donate, keep_unused=True)

    def fn(in_maps):
        concat_in = [
            np.concatenate([np.asarray(in_maps[c][n]) for c in range(n_cores)], 0)
            for n in in_names
        ]
        concat_zeros = [np.zeros((n_cores * z.shape[0], *z.shape[1:]), z.dtype)
                        for z in zero_outs]
        t0 = time.perf_counter()
        out_arrs = jfn(*concat_in, *concat_zeros)
        out_arrs = [np.asarray(o) for o in out_arrs]
        dt = time.perf_counter() - t0
        return [
            {n: out_arrs[i].reshape(n_cores, *out_avals[i].shape)[c]
             for i, n in enumerate(out_names)}
            for c in range(n_cores)
        ], dt

    return fn


_CACHE = {}


def kernel(node_features, edge_features, global_features, senders, receivers,
           W_kernel, W_bias, We_kernel, We_bias, a):
    node_features = np.asarray(node_features, np.float32)
    edge_features = np.asarray(edge_features, np.float32)
    senders = np.asarray(senders, np.int32)
    receivers = np.asarray(receivers, np.int32)
    in_maps = host_prep(node_features, edge_features, senders, receivers,
                        W_kernel, W_bias, We_kernel, We_bias, a)
    if "fn" not in _CACHE:
        nc = build_nc()
        _CACHE["fn"] = _build_runner(nc, N_CORES)
    res, dt = _CACHE["fn"](in_maps)
    _CACHE["last_dt"] = dt
    # out_shard [128, bpc, 64]; global row = (c*bpc + b)*128 + p
    full = np.concatenate(
        [r["out_shard"].transpose(1, 0, 2).reshape(BPC_FULL * 128, EMBED)
         for r in res], axis=0)
    return full[:N_NODES].astype(np.float32)
